# revision 27
# baseline (speedup 1.0000x reference)
"""CASSI adjoint (gather shifted bands + mask) as a Bass/Tile SPMD kernel
on 8 Trainium2 NeuronCores.

Reference computation (shapes hardcoded for H=W=1024, L=28, PAD=32):
    out[0, l, h, w] = y_1hw[0, dy[l] + h, dx[l] + w] * mask2d[h, w]
with integer offsets dx/dy derived from phi_d_deg and s_nom on the host.

Sharding: the H (row) dimension is split across the 8 cores — every core
runs an identical program (all 28 bands, offsets baked in as compile-time
constants) over its own 128-row chunk of y/mask/out. Zero communication.

Per-core program (memory-bound; output writes dominate at ~421 GB/s):
  - y and mask are packed host-side into one [128, 2080] f32 input so the
    load is 128 descriptors instead of 256 (HWDGE descriptor feed is the
    load bottleneck), split even/odd partitions across both HWDGE rings.
  - bands are multiplied by the mask on DVE, fused into one strided
    tensor_tensor per run of bands with uniform dy / constant dx step.
  - stores stream on a single HWDGE ring (saturates ~421 GB/s); group
    sizes ramp 1,3,4,4,... so the store pipeline starts ASAP.
"""

import numpy as np

import concourse.bass as bass
import concourse.mybir as mybir
from concourse import bacc, tile
from concourse.bass_utils import run_bass_kernel_spmd

PI = 3.141592653589793

H, W, L = 1024, 1024, 28
HP, WP = 1056, 1056  # padded input extents (H+PAD, W+PAD)
NCORES = 8
RC = H // NCORES  # 128 rows per core

_cache: dict = {}


def _offsets(phi_d_deg, s_nom):
    """Integer dispersion offsets, mirroring the f32 arithmetic of the
    reference (round-half-to-even, then dynamic_slice start clamping)."""
    phi = np.float32(np.asarray(phi_d_deg, dtype=np.float32).reshape(-1)[0])
    phi_rad = np.float32(phi * np.float32(PI / 180.0))
    s = np.asarray(s_nom, dtype=np.float32)
    dx_f = (s * np.float32(np.cos(phi_rad))).astype(np.float32)
    dy_f = (s * np.float32(np.sin(phi_rad))).astype(np.float32)
    dx_f = (dx_f - dx_f.min()).astype(np.float32)
    dy_f = (dy_f - dy_f.min()).astype(np.float32)
    dx = np.round(dx_f).astype(np.int32)
    dy = np.round(dy_f).astype(np.int32)
    dx = np.clip(dx, 0, WP - W)
    dy = np.clip(dy, 0, HP - H)
    return dx, dy


def _group_schedule(n):
    """Small leading groups so the first store dispatches early, then 4s,
    with a small final group so the last store's HBM-write receipt chases
    a short drain tail."""
    sizes = [1, 3, 4, 4, 4, 4, 4, 3, 1]
    if sum(sizes) != n:
        sizes = []
        for s in (1, 3):
            if sum(sizes) + s <= n:
                sizes.append(s)
        while sum(sizes) < n:
            sizes.append(min(4, n - sum(sizes)))
    return sizes


FP16 = True  # compute products in fp16 (DVE 2x), upcast to f32 on ACT/DVE
SWSTORE = True  # store f16 products via SWDGE casting DMA (f16->f32 in flight)
SWLOAD = False  # SWDGE casting load regressed: slow completion + DMA_15 stall
HOST16 = True  # host packs [y | mask | y-shifted] as f16: halves load bytes,
               # removes every on-chip cast (first mul fires at load receipt)
HCOLS = WP + W + WP  # 3136 packed f16 columns per row


def _build(dx, dy, obufs=9):
    """Build + compile the per-core program for the given band offsets."""
    max_dy = int(dy.max())
    packed = max_dy == 0
    nc = bacc.Bacc("TRN2", target_bir_lowering=False, debug=False,
                   num_devices=NCORES)
    f32 = mybir.dt.float32
    f16 = mybir.dt.float16
    if packed:
        if FP16 and HOST16:
            ym_in = nc.dram_tensor("ym_loc", [RC, HCOLS], f16,
                                   kind="ExternalInput")
        else:
            ym_in = nc.dram_tensor("ym_loc", [RC, WP + W], f32,
                                   kind="ExternalInput")
    else:
        y_in = nc.dram_tensor("y_loc", [RC + max_dy, WP], f32,
                              kind="ExternalInput")
        m_in = nc.dram_tensor("mask_loc", [RC, W], f32, kind="ExternalInput")
    o_out = nc.dram_tensor("out_loc", [L, RC, W], f32, kind="ExternalOutput")

    sizes = _group_schedule(L)
    max_g = max(sizes)

    with tile.TileContext(nc) as tc:
        with (
            tc.tile_pool(name="singles", bufs=1) as singles,
            tc.tile_pool(name="ob", bufs=obufs) as obp,
        ):
            if packed:
                if not FP16 or not (SWLOAD or HOST16):
                    ymt = singles.tile([RC, WP + W], f32, tag="ym", name="ym")
                    nc.sync.dma_start(out=ymt[:, :], in_=ym_in[:, :])
                    ytiles = {0: ymt}
                    mask_tile, mask_col = ymt, WP
            else:
                ytiles = {}
                for d in sorted({int(v) for v in dy}):
                    yt = singles.tile([RC, WP], f32, tag=f"y{d}", name=f"y{d}")
                    nc.sync.dma_start(out=yt[:, :], in_=y_in[d : d + RC, :])
                    ytiles[d] = yt
                mt = singles.tile([RC, W], f32, tag="mask", name="mask")
                nc.scalar.dma_start(out=mt[:, :], in_=m_in[:, :])
                mask_tile, mask_col = mt, 0

            use16 = FP16 and packed
            if use16:
                # fp16 pipeline: tensor_tensor on 16-bit runs in 2x_1P mode
                # (~610ns/band vs 1224 fp32), so the DVE never paces the DMA
                # store stream.  Odd dx offsets break the 4B-alignment the
                # 2x mode needs, so keep a one-column-shifted copy of y.
                if HOST16:
                    # host already packed [y | mask | y-shifted] in f16
                    ym16 = singles.tile([RC, HCOLS], f16, tag="ym16",
                                        name="ym16")
                    nc.sync.dma_start(out=ym16[:, :], in_=ym_in[:, :])
                    ytile, ycol = ym16, 0
                    mtile, mcol = ym16, WP
                    otile, ocol = ym16, WP + W
                else:
                    y16t = singles.tile([RC, WP + W], f16, tag="y16",
                                        name="y16")
                    y16ot = singles.tile([RC, WP], f16, tag="y16o",
                                         name="y16o")
                    if SWLOAD:
                        nc.gpsimd.dma_start(out=y16t[:, :], in_=ym_in[:, :])
                        nc.scalar.copy(y16ot[:, 0:WP], y16t[:, 1 : WP + 1])
                    else:
                        # DVE does only the y-cols cast (611ns) so the first
                        # (even-dx) mul starts ASAP; ACT casts mask + shifted
                        # copy concurrently (SWDGE stores are immune to ACT's
                        # SBUF traffic, unlike HWDGE ones)
                        nc.vector.tensor_copy(y16t[:, 0:WP], ymt[:, 0:WP])
                        nc.scalar.copy(y16t[:, WP : WP + W],
                                       ymt[:, WP : WP + W])
                        nc.scalar.copy(y16ot[:, 0:WP], ymt[:, 1 : WP + 1])
                    ytile, ycol = y16t, 0
                    mtile, mcol = y16t, WP
                    otile, ocol = y16ot, 0
                m16_ap = mtile[:, mcol : mcol + W]

            # Per-band 2D tensor_muls on DVE only. Fused 3D strided TTs
            # and GpSimd co-compute both measurably slow the concurrent
            # DMA store stream (SBUF port interference) — net losses.
            sw16 = use16 and SWSTORE
            g0 = 0
            for gsz in sizes:
                odt = f16 if sw16 else f32
                ot = obp.tile([RC, max_g * W], odt, tag="obuf", name=f"ob{g0}")
                for j in range(gsz):
                    l = g0 + j
                    x0 = int(dx[l])
                    if use16:
                        # f16 ins keep compute-engine SBUF traffic minimal:
                        # it slows the concurrent DMA store stream nearly 1:1
                        if x0 % 2 == 0:
                            src = ytile[:, ycol + x0 : ycol + x0 + W]
                        else:
                            src = otile[:, ocol + x0 - 1 : ocol + x0 - 1 + W]
                        nc.vector.tensor_mul(
                            ot[:, j * W : (j + 1) * W], src, m16_ap)
                    else:
                        ysap = ytiles[int(dy[l])][:, :]
                        nc.vector.tensor_mul(
                            ot[:, j * W : (j + 1) * W],
                            ysap[:, x0 : x0 + W],
                            mask_tile[:, mask_col : mask_col + W],
                        )
                dview = o_out[g0 : g0 + gsz, :, :].rearrange("l h w -> h l w")
                sview = ot[:, : gsz * W].rearrange("h (l w) -> h l w", w=W)
                if sw16:
                    # SWDGE casting store: SBUF side reads f16 (half the AXI
                    # port bytes), SDMA upcasts to f32 on the way to HBM
                    nc.gpsimd.dma_start(out=dview, in_=sview)
                else:
                    nc.sync.dma_start(out=dview, in_=sview)
                g0 += gsz

    nc.compile()
    return nc, packed


def _run(inputs, trace=False):
    y = np.ascontiguousarray(np.asarray(inputs["y_1hw"], dtype=np.float32)[0])
    mask = np.ascontiguousarray(np.asarray(inputs["mask2d"], dtype=np.float32))
    assert y.shape == (HP, WP) and mask.shape == (H, W)
    dx, dy = _offsets(inputs["phi_d_deg"], inputs["s_nom"])
    assert len(dx) == L

    key = (tuple(dx.tolist()), tuple(dy.tolist()))
    if key not in _cache:
        _cache[key] = _build(dx, dy)
    nc, packed = _cache[key]

    max_dy = int(dy.max())
    host16 = packed and FP16 and HOST16
    if host16:
        y16 = y.astype(np.float16)
        m16 = mask.astype(np.float16)
        # y shifted one column left (pad last col; bands never read it)
        y16s = np.concatenate(
            [y16[:, 1:], np.zeros((HP, 1), dtype=np.float16)], axis=1
        )
    in_maps = []
    for c in range(NCORES):
        h0 = c * RC
        if host16:
            in_maps.append({
                "ym_loc": np.ascontiguousarray(
                    np.concatenate(
                        [y16[h0 : h0 + RC, :], m16[h0 : h0 + RC, :],
                         y16s[h0 : h0 + RC, :]], axis=1
                    )
                ),
            })
        elif packed:
            in_maps.append({
                "ym_loc": np.ascontiguousarray(
                    np.concatenate(
                        [y[h0 : h0 + RC, :], mask[h0 : h0 + RC, :]], axis=1
                    )
                ),
            })
        else:
            in_maps.append({
                "y_loc": np.ascontiguousarray(y[h0 : h0 + RC + max_dy, :]),
                "mask_loc": np.ascontiguousarray(mask[h0 : h0 + RC, :]),
            })
    res = run_bass_kernel_spmd(nc, in_maps, core_ids=list(range(NCORES)),
                               trace=trace)
    out = np.empty((1, L, H, W), dtype=np.float32)
    for c in range(NCORES):
        out[0, :, c * RC : (c + 1) * RC, :] = res.results[c]["out_loc"]
    return out, res


def kernel(**inputs) -> np.ndarray:
    out, _ = _run(inputs)
    return out



# revision 28
# speedup vs baseline: 1.1437x; 1.1437x over previous
"""CASSI adjoint (gather shifted bands + mask) as a Bass/Tile SPMD kernel
on 8 Trainium2 NeuronCores.

Reference computation (shapes hardcoded for H=W=1024, L=28, PAD=32):
    out[0, l, h, w] = y_1hw[0, dy[l] + h, dx[l] + w] * mask2d[h, w]
with integer offsets dx/dy derived from phi_d_deg and s_nom on the host.

Sharding: the H (row) dimension is split across the 8 cores — every core
runs an identical program (all 28 bands, offsets baked in as compile-time
constants) over its own 128-row chunk of y/mask/out. Zero communication.

Per-core program (memory-bound; output writes dominate at ~421 GB/s):
  - y and mask are packed host-side into one [128, 2080] f32 input so the
    load is 128 descriptors instead of 256 (HWDGE descriptor feed is the
    load bottleneck), split even/odd partitions across both HWDGE rings.
  - bands are multiplied by the mask on DVE, fused into one strided
    tensor_tensor per run of bands with uniform dy / constant dx step.
  - stores stream on a single HWDGE ring (saturates ~421 GB/s); group
    sizes ramp 1,3,4,4,... so the store pipeline starts ASAP.
"""

import numpy as np

import concourse.bass as bass
import concourse.mybir as mybir
from concourse import bacc, tile
from concourse.bass_utils import run_bass_kernel_spmd

PI = 3.141592653589793

H, W, L = 1024, 1024, 28
HP, WP = 1056, 1056  # padded input extents (H+PAD, W+PAD)
NCORES = 8
RC = H // NCORES  # 128 rows per core

_cache: dict = {}


def _offsets(phi_d_deg, s_nom):
    """Integer dispersion offsets, mirroring the f32 arithmetic of the
    reference (round-half-to-even, then dynamic_slice start clamping)."""
    phi = np.float32(np.asarray(phi_d_deg, dtype=np.float32).reshape(-1)[0])
    phi_rad = np.float32(phi * np.float32(PI / 180.0))
    s = np.asarray(s_nom, dtype=np.float32)
    dx_f = (s * np.float32(np.cos(phi_rad))).astype(np.float32)
    dy_f = (s * np.float32(np.sin(phi_rad))).astype(np.float32)
    dx_f = (dx_f - dx_f.min()).astype(np.float32)
    dy_f = (dy_f - dy_f.min()).astype(np.float32)
    dx = np.round(dx_f).astype(np.int32)
    dy = np.round(dy_f).astype(np.int32)
    dx = np.clip(dx, 0, WP - W)
    dy = np.clip(dy, 0, HP - H)
    return dx, dy


def _group_schedule(n):
    """Small leading groups so the first store dispatches early, then 4s,
    with a small final group so the last store's HBM-write receipt chases
    a short drain tail."""
    sizes = [1, 3, 4, 4, 4, 4, 4, 3, 1]
    if sum(sizes) != n:
        sizes = []
        for s in (1, 3):
            if sum(sizes) + s <= n:
                sizes.append(s)
        while sum(sizes) < n:
            sizes.append(min(4, n - sum(sizes)))
    return sizes


FP16 = True  # compute products in fp16 (DVE 2x), upcast to f32 on ACT/DVE
SWSTORE = True  # store f16 products via SWDGE casting DMA (f16->f32 in flight)
SWLOAD = False  # SWDGE casting load regressed: slow completion + DMA_15 stall
HOST16 = True  # host packs [y | mask | y-shifted] as f16: halves load bytes,
               # removes every on-chip cast (first mul fires at load receipt)
HCOLS = WP + W + WP  # 3136 packed f16 columns per row


def _build(dx, dy, obufs=9):
    """Build + compile the per-core program for the given band offsets."""
    max_dy = int(dy.max())
    packed = max_dy == 0
    nc = bacc.Bacc("TRN2", target_bir_lowering=False, debug=False,
                   num_devices=NCORES)
    f32 = mybir.dt.float32
    f16 = mybir.dt.float16
    if packed:
        if FP16 and HOST16:
            ym_in = nc.dram_tensor("ym_loc", [RC, HCOLS], f16,
                                   kind="ExternalInput")
        else:
            ym_in = nc.dram_tensor("ym_loc", [RC, WP + W], f32,
                                   kind="ExternalInput")
    else:
        y_in = nc.dram_tensor("y_loc", [RC + max_dy, WP], f32,
                              kind="ExternalInput")
        m_in = nc.dram_tensor("mask_loc", [RC, W], f32, kind="ExternalInput")
    o_out = nc.dram_tensor("out_loc", [L, RC, W], f32, kind="ExternalOutput")

    sizes = _group_schedule(L)
    max_g = max(sizes)

    with tile.TileContext(nc) as tc:
        with (
            tc.tile_pool(name="singles", bufs=1) as singles,
            tc.tile_pool(name="ob", bufs=obufs) as obp,
        ):
            if packed:
                if not FP16 or not (SWLOAD or HOST16):
                    ymt = singles.tile([RC, WP + W], f32, tag="ym", name="ym")
                    nc.sync.dma_start(out=ymt[:, :], in_=ym_in[:, :])
                    ytiles = {0: ymt}
                    mask_tile, mask_col = ymt, WP
            else:
                ytiles = {}
                for d in sorted({int(v) for v in dy}):
                    yt = singles.tile([RC, WP], f32, tag=f"y{d}", name=f"y{d}")
                    nc.sync.dma_start(out=yt[:, :], in_=y_in[d : d + RC, :])
                    ytiles[d] = yt
                mt = singles.tile([RC, W], f32, tag="mask", name="mask")
                nc.scalar.dma_start(out=mt[:, :], in_=m_in[:, :])
                mask_tile, mask_col = mt, 0

            use16 = FP16 and packed
            if use16:
                # fp16 pipeline: tensor_tensor on 16-bit runs in 2x_1P mode
                # (~610ns/band vs 1224 fp32), so the DVE never paces the DMA
                # store stream.  Odd dx offsets break the 4B-alignment the
                # 2x mode needs, so keep a one-column-shifted copy of y.
                if HOST16:
                    # pad so the ob pool lands at the same SBUF base as the
                    # R6 layout: at other bases the SWDGE store stream stalls
                    # on SDMA engine 15 (~181ns vs 148ns per packet)
                    singles.tile([RC, 4160], f16, tag="pad", name="pad")
                    # host already packed [y | mask | y-shifted] in f16
                    ym16 = singles.tile([RC, HCOLS], f16, tag="ym16",
                                        name="ym16")
                    nc.sync.dma_start(out=ym16[:, :], in_=ym_in[:, :])
                    ytile, ycol = ym16, 0
                    mtile, mcol = ym16, WP
                    otile, ocol = ym16, WP + W
                else:
                    y16t = singles.tile([RC, WP + W], f16, tag="y16",
                                        name="y16")
                    y16ot = singles.tile([RC, WP], f16, tag="y16o",
                                         name="y16o")
                    if SWLOAD:
                        nc.gpsimd.dma_start(out=y16t[:, :], in_=ym_in[:, :])
                        nc.scalar.copy(y16ot[:, 0:WP], y16t[:, 1 : WP + 1])
                    else:
                        # DVE does only the y-cols cast (611ns) so the first
                        # (even-dx) mul starts ASAP; ACT casts mask + shifted
                        # copy concurrently (SWDGE stores are immune to ACT's
                        # SBUF traffic, unlike HWDGE ones)
                        nc.vector.tensor_copy(y16t[:, 0:WP], ymt[:, 0:WP])
                        nc.scalar.copy(y16t[:, WP : WP + W],
                                       ymt[:, WP : WP + W])
                        nc.scalar.copy(y16ot[:, 0:WP], ymt[:, 1 : WP + 1])
                    ytile, ycol = y16t, 0
                    mtile, mcol = y16t, WP
                    otile, ocol = y16ot, 0
                m16_ap = mtile[:, mcol : mcol + W]

            # Per-band 2D tensor_muls on DVE only. Fused 3D strided TTs
            # and GpSimd co-compute both measurably slow the concurrent
            # DMA store stream (SBUF port interference) — net losses.
            sw16 = use16 and SWSTORE
            g0 = 0
            for gsz in sizes:
                odt = f16 if sw16 else f32
                ot = obp.tile([RC, max_g * W], odt, tag="obuf", name=f"ob{g0}")
                for j in range(gsz):
                    l = g0 + j
                    x0 = int(dx[l])
                    if use16:
                        # f16 ins keep compute-engine SBUF traffic minimal:
                        # it slows the concurrent DMA store stream nearly 1:1
                        if x0 % 2 == 0:
                            src = ytile[:, ycol + x0 : ycol + x0 + W]
                        else:
                            src = otile[:, ocol + x0 - 1 : ocol + x0 - 1 + W]
                        nc.vector.tensor_mul(
                            ot[:, j * W : (j + 1) * W], src, m16_ap)
                    else:
                        ysap = ytiles[int(dy[l])][:, :]
                        nc.vector.tensor_mul(
                            ot[:, j * W : (j + 1) * W],
                            ysap[:, x0 : x0 + W],
                            mask_tile[:, mask_col : mask_col + W],
                        )
                dview = o_out[g0 : g0 + gsz, :, :].rearrange("l h w -> h l w")
                sview = ot[:, : gsz * W].rearrange("h (l w) -> h l w", w=W)
                if sw16:
                    # SWDGE casting store: SBUF side reads f16 (half the AXI
                    # port bytes), SDMA upcasts to f32 on the way to HBM
                    nc.gpsimd.dma_start(out=dview, in_=sview)
                else:
                    nc.sync.dma_start(out=dview, in_=sview)
                g0 += gsz

    nc.compile()
    return nc, packed


def _run(inputs, trace=False):
    y = np.ascontiguousarray(np.asarray(inputs["y_1hw"], dtype=np.float32)[0])
    mask = np.ascontiguousarray(np.asarray(inputs["mask2d"], dtype=np.float32))
    assert y.shape == (HP, WP) and mask.shape == (H, W)
    dx, dy = _offsets(inputs["phi_d_deg"], inputs["s_nom"])
    assert len(dx) == L

    key = (tuple(dx.tolist()), tuple(dy.tolist()))
    if key not in _cache:
        _cache[key] = _build(dx, dy)
    nc, packed = _cache[key]

    max_dy = int(dy.max())
    host16 = packed and FP16 and HOST16
    if host16:
        y16 = y.astype(np.float16)
        m16 = mask.astype(np.float16)
        # y shifted one column left (pad last col; bands never read it)
        y16s = np.concatenate(
            [y16[:, 1:], np.zeros((HP, 1), dtype=np.float16)], axis=1
        )
    in_maps = []
    for c in range(NCORES):
        h0 = c * RC
        if host16:
            in_maps.append({
                "ym_loc": np.ascontiguousarray(
                    np.concatenate(
                        [y16[h0 : h0 + RC, :], m16[h0 : h0 + RC, :],
                         y16s[h0 : h0 + RC, :]], axis=1
                    )
                ),
            })
        elif packed:
            in_maps.append({
                "ym_loc": np.ascontiguousarray(
                    np.concatenate(
                        [y[h0 : h0 + RC, :], mask[h0 : h0 + RC, :]], axis=1
                    )
                ),
            })
        else:
            in_maps.append({
                "y_loc": np.ascontiguousarray(y[h0 : h0 + RC + max_dy, :]),
                "mask_loc": np.ascontiguousarray(mask[h0 : h0 + RC, :]),
            })
    res = run_bass_kernel_spmd(nc, in_maps, core_ids=list(range(NCORES)),
                               trace=trace)
    out = np.empty((1, L, H, W), dtype=np.float32)
    for c in range(NCORES):
        out[0, :, c * RC : (c + 1) * RC, :] = res.results[c]["out_loc"]
    return out, res


def kernel(**inputs) -> np.ndarray:
    out, _ = _run(inputs)
    return out

